# revision 1
# baseline (speedup 1.0000x reference)
"""Two-layer GAT (PyG GATConv semantics) on 8 Trainium2 NeuronCores.

Strategy (graph/data parallel, per sharding hint):
  - Destination nodes are range-sharded across the 8 cores (3200 dsts/core,
    nodes padded 25000 -> 25600).
  - Every core redundantly computes the full layer-1 node features
    h = x @ W1ext (W1ext also folds the attention projections: es = x @ (W1 @
    a_src), ed = x @ (W1 @ a_dst)), writes them as a row table in its own HBM.
  - Edges (with self loops) are sorted by dst on the host and bucketed per
    128-dst tile; each tile's edge list is padded to NG groups of 128 edges.
  - Per edge group the core dma_gathers the 128 source rows, computes the
    edge logits e = leakyrelu(es[src] + ed[dst]), p = exp(e), and reduces
    both the attention-weighted feature sum and the softmax denominator with
    one-hot matmuls on the PE (host-precomputed one-hot masks, both
    orientations, streamed from HBM).  Softmax max-subtraction is skipped
    (shift invariance; logits are O(1) so exp cannot overflow).
  - Layer 2 repeats the same scheme on h2 = ELU(out1/denom); the layer-2
    row table (h2 @ W2ext plus es2/ed2 columns) is exchanged between cores
    with an AllGather collective.

Row table layouts (bf16 elements):
  L1 row (576): [h interleaved: for head h: 128 features then 1.0] (516)
                | es float32 x4 (elems 516:524) | ed float32 x4 (524:532)
                | pad (532:576)
  The interleaved 1.0 columns let one tensor_tensor multiply produce both
  p*h and p itself, so a single one-hot matmul yields the weighted feature
  sum and the softmax denominator together.
  L2 row (128): [h2' (64) | 1.0 (64) | pad (65) | es2 f32 (66:68)
                | ed2 f32 (68:70) | pad]
"""

import sys

for _p in ("/opt/trn_rl_repo",):
    if _p not in sys.path:
        sys.path.insert(0, _p)

import numpy as np
import ml_dtypes

import concourse.bacc as bacc
import concourse.bass as bass
import concourse.mybir as mybir
import concourse.tile as tile
from concourse import library_config
from concourse._compat import axon_active
from concourse.bass_utils import run_bass_kernel_spmd
from concourse.masks import make_identity

BF16 = ml_dtypes.bfloat16
F32 = mybir.dt.float32
BF = mybir.dt.bfloat16
P = 128
NCORES = 8


class GATConfig:
    def __init__(self, n, in_ch, hid, heads, out_ch, neg_slope, ng):
        self.N = n
        self.NPAD = -(-n // (P * NCORES)) * (P * NCORES)
        self.SHARD = self.NPAD // NCORES
        self.T = self.SHARD // P              # dst tiles per core
        self.NT = self.NPAD // P              # node tiles (phase A)
        self.IN_CH = in_ch                    # must be multiple of 128
        self.KIN = in_ch // P
        self.HID = hid                        # 128
        self.HEADS = heads                    # 4
        self.OUT_CH = out_ch                  # 64
        self.NEG = neg_slope
        self.NG = ng                          # edge groups per dst tile
        self.H1 = heads * hid                 # 512
        # L1 row layout
        self.R1_HE = heads * (hid + 1)        # 516 interleaved h+ones
        self.R1_ES = self.R1_HE               # es f32 at elems 516:524
        self.R1_ED = self.R1_HE + 8           # ed f32 at elems 524:532
        self.ROW1 = -(-(self.R1_HE + 16) * 2 // 256) * 128  # pad bytes->%256
        # L2 row layout
        self.R2_ES = out_ch + 2               # es2 f32 at elems 66:68
        self.R2_ED = out_ch + 4               # ed2 f32 at elems 68:70
        self.ROW2 = -(-(out_ch + 6) * 2 // 256) * 128
        self.KH1 = self.H1 // P               # 4


def _wrap_idx(flat):
    """int16 flat index list -> [128, len/16] wrapped layout for dma_gather."""
    n = len(flat)
    assert n % 16 == 0
    w = np.asarray(flat, np.int16).reshape(n // 16, 16).T  # [16, n/16]
    return np.tile(w, (8, 1))                              # [128, n/16]


def host_prep(cfg, x, edge_index, W1, a_src1, a_dst1, b1, W2, a_src2, a_dst2, b2):
    """Build all per-core input arrays. Returns (in_maps, meta)."""
    N, NPAD = cfg.N, cfg.NPAD
    H, C, OC = cfg.HEADS, cfg.HID, cfg.OUT_CH

    # --- weights ---------------------------------------------------------
    W1 = np.asarray(W1, np.float32)
    W2 = np.asarray(W2, np.float32)
    a_src1 = np.asarray(a_src1, np.float32)
    a_dst1 = np.asarray(a_dst1, np.float32)
    a_src2 = np.asarray(a_src2, np.float32)
    a_dst2 = np.asarray(a_dst2, np.float32)
    w1ext = np.zeros((cfg.IN_CH, cfg.H1 + 8), np.float32)
    w1ext[:, : cfg.H1] = W1
    for h in range(H):
        w1ext[:, cfg.H1 + h] = W1[:, h * C : (h + 1) * C] @ a_src1[h]
        w1ext[:, cfg.H1 + 4 + h] = W1[:, h * C : (h + 1) * C] @ a_dst1[h]
    # -> [128, KIN, 520]
    w1eh = np.ascontiguousarray(
        w1ext.reshape(cfg.KIN, P, cfg.H1 + 8).transpose(1, 0, 2)
    ).astype(BF16)

    w2ext = np.zeros((cfg.H1, OC + 2), np.float32)
    w2ext[:, :OC] = W2
    w2ext[:, OC] = W2 @ a_src2[0]
    w2ext[:, OC + 1] = W2 @ a_dst2[0]
    w2eh = np.ascontiguousarray(
        w2ext.reshape(cfg.KH1, P, OC + 2).transpose(1, 0, 2)
    ).astype(BF16)

    # --- x, transposed+tiled for lhsT: XTH[k, nt*KIN*128 + j*128 + m] ----
    xp = np.zeros((NPAD, cfg.IN_CH), np.float32)
    xp[:N] = np.asarray(x, np.float32)
    # [nt, m, j, k] -> [k, nt, j, m]
    xth = np.ascontiguousarray(
        xp.reshape(cfg.NT, P, cfg.KIN, P).transpose(3, 0, 2, 1)
    ).reshape(P, cfg.NT * cfg.KIN * P).astype(BF16)

    # --- edges -----------------------------------------------------------
    ei = np.asarray(edge_index, np.int64)
    loop = np.arange(N, dtype=np.int64)
    src = np.concatenate([ei[0], loop])
    dst = np.concatenate([ei[1], loop])
    order = np.argsort(dst, kind="stable")
    src_s = src[order].astype(np.int32)
    dst_s = dst[order].astype(np.int32)
    gtiles = cfg.NT  # global dst tiles (= NCORES * T)
    counts = np.bincount(dst_s // P, minlength=gtiles)
    ng = int(np.ceil(counts.max() / P)) if counts.max() else 1
    assert ng <= cfg.NG, f"data needs NG={ng} > configured {cfg.NG}"
    NG = cfg.NG
    ET = NG * P
    starts = np.concatenate([[0], np.cumsum(counts)])

    gidx = np.zeros((NCORES, cfg.T, P, NG * 8), np.int16)
    gidxed = np.zeros((NCORES, cfg.T, P, 8), np.int16)
    maskpair = np.zeros((NCORES, cfg.T, NG, P, 2 * P), BF16)
    eye = np.arange(P, dtype=np.int32)
    for gt in range(gtiles):
        c, t = divmod(gt, cfg.T)
        lo, hi = starts[gt], starts[gt + 1]
        k = hi - lo
        idx = np.zeros(ET, np.int32)
        idx[:k] = src_s[lo:hi]
        gidx[c, t] = _wrap_idx(idx.astype(np.int16))
        gidxed[c, t] = _wrap_idx((gt * P + eye).astype(np.int16))
        dl = np.full(ET, -1, np.int32)
        dl[:k] = dst_s[lo:hi] - gt * P
        m = dl.reshape(NG, P, 1) == eye.reshape(1, 1, P)  # [NG, e, d]
        maskpair[c, t, :, :, :P] = m
        maskpair[c, t, :, :, P:] = m.transpose(0, 2, 1)

    in_maps = []
    for c in range(NCORES):
        in_maps.append(
            {
                "xth": xth,
                "w1eh": w1eh,
                "w2eh": w2eh,
                "gidx": gidx[c].reshape(cfg.T * P, NG * 8),
                "gidxed": gidxed[c].reshape(cfg.T * P, 8),
                "maskpair": maskpair[c].reshape(cfg.T * NG * P, 2 * P),
            }
        )
    meta = {
        "b1_nonzero": bool(np.any(np.asarray(b1))),
        "b2_nonzero": bool(np.any(np.asarray(b2))),
        "b1": np.asarray(b1, np.float32),
        "b2": np.asarray(b2, np.float32),
    }
    return in_maps, meta


def build_program(cfg, meta, phases="ABCD"):
    under_axon = axon_active()
    nc = bacc.Bacc(
        "TRN2",
        target_bir_lowering=False,
        debug=not under_axon,
        num_devices=NCORES,
        # default 16 KiB ring caps one SWDGE op at ~1024 descriptors; our
        # row gathers are chunked to 8 groups (1024 rows) per call and the
        # bigger carveout keeps several calls in flight
        dynamic_dma_scratch_size=65536,
    )
    H, C, OC, NG, T = cfg.HEADS, cfg.HID, cfg.OUT_CH, cfg.NG, cfg.T
    H1, KIN, KH1 = cfg.H1, cfg.KIN, cfg.KH1

    # I/O ------------------------------------------------------------------
    xth_d = nc.dram_tensor("xth", [P, cfg.NT * KIN * P], BF, kind="ExternalInput")
    w1eh_d = nc.dram_tensor("w1eh", [P, KIN, H1 + 8], BF, kind="ExternalInput")
    w2eh_d = nc.dram_tensor("w2eh", [P, KH1, OC + 2], BF, kind="ExternalInput")
    gidx_d = nc.dram_tensor("gidx", [T * P, NG * 8], mybir.dt.int16, kind="ExternalInput")
    gidxed_d = nc.dram_tensor("gidxed", [T * P, 8], mybir.dt.int16, kind="ExternalInput")
    maskp_d = nc.dram_tensor(
        "maskpair", [T * NG * P, 2 * P], BF, kind="ExternalInput"
    )
    out_d = nc.dram_tensor("out", [cfg.SHARD, OC], F32, kind="ExternalOutput")

    table1 = nc.dram_tensor("table1", [cfg.NPAD, cfg.ROW1], BF)
    t2shard = nc.dram_tensor("t2shard", [cfg.SHARD, cfg.ROW2], BF)
    t2full = nc.dram_tensor("t2full", [cfg.NPAD, cfg.ROW2], BF, addr_space="Shared")

    # biases host-replicated across partitions (DVE lanes cannot read a
    # different partition, so a [1, n] row cannot be broadcast on-chip)
    if meta["b1_nonzero"]:
        b1_d = nc.dram_tensor("b1", [P, H1], F32, kind="ExternalInput")
    if meta["b2_nonzero"]:
        b2_d = nc.dram_tensor("b2", [P, OC], F32, kind="ExternalInput")

    with tile.TileContext(nc) as tc:
        nc.gpsimd.load_library(library_config.mlp)

        # persistent SBUF
        with tc.tile_pool(name="persist", bufs=1) as pp:
            w1eh = pp.tile([P, KIN, H1 + 8], BF)
            nc.sync.dma_start(out=w1eh[:], in_=w1eh_d[:])
            w2eh = pp.tile([P, KH1, OC + 2], BF)
            nc.sync.dma_start(out=w2eh[:], in_=w2eh_d[:])
            gidx = pp.tile([P, T, NG * 8], mybir.dt.int16)
            nc.sync.dma_start(
                out=gidx[:],
                in_=gidx_d[:].rearrange("(t p) s -> p t s", p=P),
            )
            gidxed = pp.tile([P, T, 8], mybir.dt.int16)
            nc.sync.dma_start(
                out=gidxed[:], in_=gidxed_d[:].rearrange("(t p) s -> p t s", p=P)
            )
            ident = pp.tile([P, P], BF)
            make_identity(nc, ident[:])
            if meta["b1_nonzero"]:
                b1_sb = pp.tile([P, H1], F32)
                nc.sync.dma_start(out=b1_sb[:], in_=b1_d[:])
            if meta["b2_nonzero"]:
                b2_sb = pp.tile([P, OC], F32)
                nc.sync.dma_start(out=b2_sb[:], in_=b2_d[:])

            # ---------------- Phase A: h table ---------------------------
            if "A" not in phases:
                raise ValueError("phase A required")
            with (
                tc.tile_pool(name="xth_pool", bufs=1) as xp_pool,
                tc.tile_pool(name="pa_sb", bufs=3) as pa_sb,
                tc.tile_pool(name="pa_ps", bufs=2, space="PSUM") as pa_ps,
            ):
                xth = xp_pool.tile([P, cfg.NT * KIN * P], BF)
                nc.sync.dma_start(out=xth[:], in_=xth_d[:])
                for nt in range(cfg.NT):
                    hps = pa_ps.tile([P, H1], F32, tag="hps")
                    eps = pa_ps.tile([P, 8], F32, tag="eps")
                    for j in range(KIN):
                        lhs = xth[:, (nt * KIN + j) * P : (nt * KIN + j + 1) * P]
                        nc.tensor.matmul(
                            out=hps[:],
                            lhsT=lhs,
                            rhs=w1eh[:, j, 0:H1],
                            start=(j == 0),
                            stop=(j == KIN - 1),
                        )
                        nc.tensor.matmul(
                            out=eps[:],
                            lhsT=lhs,
                            rhs=w1eh[:, j, H1 : H1 + 8],
                            start=(j == 0),
                            stop=(j == KIN - 1),
                        )
                    stage = pa_sb.tile([P, cfg.ROW1], BF, tag="stage")
                    # interleaved h (cast to bf16)
                    nc.scalar.activation(
                        out=stage[:, 0 : cfg.R1_HE].rearrange(
                            "p (h c) -> p h c", c=C + 1
                        )[:, :, 0:C],
                        in_=hps[:].rearrange("p (h c) -> p h c", c=C),
                        func=mybir.ActivationFunctionType.Copy,
                    )
                    # ones columns
                    nc.vector.memset(
                        stage[:, C : cfg.R1_HE : C + 1], 1.0
                    )
                    # es/ed as f32 (8 values = 16 bf16 slots)
                    nc.vector.tensor_copy(
                        out=stage[:, cfg.R1_ES : cfg.R1_ES + 16].bitcast(F32),
                        in_=eps[:],
                    )
                    if cfg.ROW1 > cfg.R1_ED + 8:
                        nc.vector.memset(stage[:, cfg.R1_ED + 8 :], 0.0)
                    nc.sync.dma_start(
                        out=table1[nt * P : (nt + 1) * P, :], in_=stage[:]
                    )

            # ---------------- Phase B+: per dst-tile ---------------------
            with (
                tc.tile_pool(name="gat_sb", bufs=2) as gsb,
                tc.tile_pool(name="mask_sb", bufs=4) as msb,
                tc.tile_pool(name="small_sb", bufs=4) as ssb,
                tc.tile_pool(name="gat_ps", bufs=2, space="PSUM") as gps,
                tc.tile_pool(name="acc_ps", bufs=1, space="PSUM") as aps,
            ):
                # ---- layer 1 aggregation + table2 rows ----
                for t in range(T if "B" in phases else 0):
                    hg = gsb.tile([P, NG, cfg.ROW1], BF, tag="hg")
                    for c0 in range(0, NG, 8):
                        gch = min(8, NG - c0)
                        nc.gpsimd.dma_gather(
                            out_ap=hg[:, c0 : c0 + gch, :],
                            in_ap=table1[:],
                            idxs_ap=gidx[:, t, c0 * 8 : (c0 + gch) * 8],
                            num_idxs=gch * P,
                            num_idxs_reg=gch * P,
                            elem_size=cfg.ROW1,
                        )
                    edrow = ssb.tile([P, 1, cfg.ROW1], BF, tag="edrow")
                    nc.gpsimd.dma_gather(
                        out_ap=edrow[:],
                        in_ap=table1[:],
                        idxs_ap=gidxed[:, t, :],
                        num_idxs=P,
                        num_idxs_reg=P,
                        elem_size=cfg.ROW1,
                    )
                    edbf = ssb.tile([P, H], BF, tag="edbf")
                    nc.vector.tensor_copy(
                        out=edbf[:],
                        in_=edrow[:, 0, cfg.R1_ED : cfg.R1_ED + 8].bitcast(F32),
                    )
                    ps1 = aps.tile([P, 2 * (C + 1)], F32, tag="ps1")
                    ps2 = aps.tile([P, 2 * (C + 1)], F32, tag="ps2")
                    for g in range(NG):
                        mp = msb.tile([P, 2 * P], BF, tag="mp")
                        nc.sync.dma_start(
                            out=mp[:],
                            in_=maskp_d[(t * NG + g) * P : (t * NG + g + 1) * P, :],
                        )
                        mask = mp[:, 0:P]
                        maskT = mp[:, P : 2 * P]
                        edps = gps.tile([P, H], F32, tag="edps")
                        nc.tensor.matmul(
                            out=edps[:], lhsT=maskT, rhs=edbf[:], start=True, stop=True
                        )
                        elog = ssb.tile([P, H], F32, tag="elog")
                        nc.vector.tensor_tensor(
                            out=elog[:],
                            in0=hg[:, g, cfg.R1_ES : cfg.R1_ES + 8].bitcast(F32),
                            in1=edps[:],
                            op=mybir.AluOpType.add,
                        )
                        esc = ssb.tile([P, H], F32, tag="esc")
                        nc.vector.tensor_scalar_mul(
                            out=esc[:], in0=elog[:], scalar1=cfg.NEG
                        )
                        nc.vector.tensor_tensor(
                            out=elog[:],
                            in0=elog[:],
                            in1=esc[:],
                            op=mybir.AluOpType.max,
                        )
                        pbf = ssb.tile([P, H], BF, tag="pbf")
                        nc.scalar.activation(
                            out=pbf[:],
                            in_=elog[:],
                            func=mybir.ActivationFunctionType.Exp,
                        )
                        ph = gsb.tile([P, cfg.R1_HE], BF, tag="ph")
                        nc.vector.tensor_tensor(
                            out=ph[:].rearrange("p (h c) -> p h c", c=C + 1),
                            in0=hg[:, g, 0 : cfg.R1_HE].rearrange(
                                "p (h c) -> p h c", c=C + 1
                            ),
                            in1=pbf[:].to_broadcast([P, H, C + 1]),
                            op=mybir.AluOpType.mult,
                        )
                        nc.tensor.matmul(
                            out=ps1[:],
                            lhsT=mask,
                            rhs=ph[:, 0 : 2 * (C + 1)],
                            start=(g == 0),
                            stop=(g == NG - 1),
                        )
                        nc.tensor.matmul(
                            out=ps2[:],
                            lhsT=mask,
                            rhs=ph[:, 2 * (C + 1) : 4 * (C + 1)],
                            start=(g == 0),
                            stop=(g == NG - 1),
                        )
                    # denominators -> reciprocal
                    dn = ssb.tile([P, H], F32, tag="dn")
                    nc.vector.tensor_copy(out=dn[:, 0:2], in_=ps1[:, C :: C + 1])
                    nc.vector.tensor_copy(out=dn[:, 2:4], in_=ps2[:, C :: C + 1])
                    nc.vector.tensor_scalar_add(out=dn[:], in0=dn[:], scalar1=1e-16)
                    rc = ssb.tile([P, H], F32, tag="rc")
                    nc.vector.reciprocal(out=rc[:], in_=dn[:])
                    # v = out1 * recip  [P, 512] f32
                    v = gsb.tile([P, H1], F32, tag="v")
                    nc.vector.tensor_tensor(
                        out=v[:, 0 : 2 * C].rearrange("p (h c) -> p h c", c=C),
                        in0=ps1[:].rearrange("p (h c) -> p h c", c=C + 1)[:, :, 0:C],
                        in1=rc[:, 0:2].to_broadcast([P, 2, C]),
                        op=mybir.AluOpType.mult,
                    )
                    nc.vector.tensor_tensor(
                        out=v[:, 2 * C : 4 * C].rearrange("p (h c) -> p h c", c=C),
                        in0=ps2[:].rearrange("p (h c) -> p h c", c=C + 1)[:, :, 0:C],
                        in1=rc[:, 2:4].to_broadcast([P, 2, C]),
                        op=mybir.AluOpType.mult,
                    )
                    if meta["b1_nonzero"]:
                        nc.vector.tensor_tensor(
                            out=v[:],
                            in0=v[:],
                            in1=b1_sb[:],
                            op=mybir.AluOpType.add,
                        )
                    # ELU -> bf16
                    rneg = gsb.tile([P, H1], F32, tag="rneg")
                    nc.scalar.activation(
                        out=rneg[:],
                        in_=v[:],
                        func=mybir.ActivationFunctionType.Relu,
                        scale=-1.0,
                    )
                    sexp = gsb.tile([P, H1], F32, tag="sexp")
                    nc.scalar.activation(
                        out=sexp[:],
                        in_=rneg[:],
                        func=mybir.ActivationFunctionType.Exp,
                        scale=-1.0,
                    )
                    rpos = gsb.tile([P, H1], F32, tag="rpos")
                    nc.scalar.activation(
                        out=rpos[:],
                        in_=v[:],
                        func=mybir.ActivationFunctionType.Relu,
                    )
                    nc.vector.tensor_tensor(
                        out=sexp[:],
                        in0=rpos[:],
                        in1=sexp[:],
                        op=mybir.AluOpType.add,
                    )
                    h2bf = gsb.tile([P, H1], BF, tag="h2bf")
                    nc.vector.tensor_scalar_add(out=h2bf[:], in0=sexp[:], scalar1=-1.0)
                    # transpose h2 -> 4x [128,128], then W2ext matmul
                    h2p = aps.tile([P, OC + 2], F32, tag="h2p")
                    for j in range(KH1):
                        tp = aps.tile([P, P], BF, tag="tp")
                        nc.tensor.transpose(
                            out=tp[:], in_=h2bf[:, j * P : (j + 1) * P], identity=ident[:]
                        )
                        h2t = ssb.tile([P, P], BF, tag="h2t")
                        nc.vector.tensor_copy(out=h2t[:], in_=tp[:])
                        nc.tensor.matmul(
                            out=h2p[:],
                            lhsT=h2t[:],
                            rhs=w2eh[:, j, :],
                            start=(j == 0),
                            stop=(j == KH1 - 1),
                        )
                    stage2 = ssb.tile([P, cfg.ROW2], BF, tag="stage2")
                    nc.scalar.activation(
                        out=stage2[:, 0:OC],
                        in_=h2p[:, 0:OC],
                        func=mybir.ActivationFunctionType.Copy,
                    )
                    nc.vector.memset(stage2[:, OC : OC + 2], 0.0)
                    nc.vector.memset(stage2[:, OC : OC + 1], 1.0)
                    nc.vector.tensor_copy(
                        out=stage2[:, cfg.R2_ES : cfg.R2_ES + 4].bitcast(F32),
                        in_=h2p[:, OC : OC + 2],
                    )
                    if cfg.ROW2 > cfg.R2_ED + 4:
                        nc.vector.memset(stage2[:, cfg.R2_ED + 4 :], 0.0)
                    nc.sync.dma_start(
                        out=t2shard[t * P : (t + 1) * P, :], in_=stage2[:]
                    )

                # ---- all-gather the layer-2 table ----
                if "C" in phases:
                    nc.gpsimd.collective_compute(
                        "AllGather",
                        mybir.AluOpType.bypass,
                        replica_groups=[list(range(NCORES))],
                        ins=[t2shard.ap().opt()],
                        outs=[t2full.ap().opt()],
                    )

                # ---- layer 2 aggregation -> output ----
                for t in range(T if "D" in phases else 0):
                    hg2 = gsb.tile([P, NG, cfg.ROW2], BF, tag="hg2")
                    for c0 in range(0, NG, 8):
                        gch = min(8, NG - c0)
                        nc.gpsimd.dma_gather(
                            out_ap=hg2[:, c0 : c0 + gch, :],
                            in_ap=t2full[:],
                            idxs_ap=gidx[:, t, c0 * 8 : (c0 + gch) * 8],
                            num_idxs=gch * P,
                            num_idxs_reg=gch * P,
                            elem_size=cfg.ROW2,
                        )
                    ed2row = ssb.tile([P, 1, cfg.ROW2], BF, tag="ed2row")
                    nc.gpsimd.dma_gather(
                        out_ap=ed2row[:],
                        in_ap=t2full[:],
                        idxs_ap=gidxed[:, t, :],
                        num_idxs=P,
                        num_idxs_reg=P,
                        elem_size=cfg.ROW2,
                    )
                    ed2bf = ssb.tile([P, 1], BF, tag="ed2bf")
                    nc.vector.tensor_copy(
                        out=ed2bf[:],
                        in_=ed2row[:, 0, cfg.R2_ED : cfg.R2_ED + 2].bitcast(F32),
                    )
                    ps3 = aps.tile([P, OC + 1], F32, tag="ps3")
                    for g in range(NG):
                        mp = msb.tile([P, 2 * P], BF, tag="mp")
                        nc.sync.dma_start(
                            out=mp[:],
                            in_=maskp_d[(t * NG + g) * P : (t * NG + g + 1) * P, :],
                        )
                        mask = mp[:, 0:P]
                        maskT = mp[:, P : 2 * P]
                        edps2_t = gps.tile([P, H], F32, tag="edps", name="edps2")
                        edps2 = edps2_t[:, 0:1]
                        nc.tensor.matmul(
                            out=edps2[:], lhsT=maskT, rhs=ed2bf[:], start=True, stop=True
                        )
                        elog2 = ssb.tile([P, 1], F32, tag="elog2")
                        nc.vector.tensor_tensor(
                            out=elog2[:],
                            in0=hg2[:, g, cfg.R2_ES : cfg.R2_ES + 2].bitcast(F32),
                            in1=edps2[:],
                            op=mybir.AluOpType.add,
                        )
                        esc2 = ssb.tile([P, 1], F32, tag="esc2")
                        nc.vector.tensor_scalar_mul(
                            out=esc2[:], in0=elog2[:], scalar1=cfg.NEG
                        )
                        nc.vector.tensor_tensor(
                            out=elog2[:],
                            in0=elog2[:],
                            in1=esc2[:],
                            op=mybir.AluOpType.max,
                        )
                        p2bf = ssb.tile([P, 1], BF, tag="p2bf")
                        nc.scalar.activation(
                            out=p2bf[:],
                            in_=elog2[:],
                            func=mybir.ActivationFunctionType.Exp,
                        )
                        ph2 = ssb.tile([P, OC + 1], BF, tag="ph2")
                        nc.vector.tensor_tensor(
                            out=ph2[:],
                            in0=hg2[:, g, 0 : OC + 1],
                            in1=p2bf[:].to_broadcast([P, OC + 1]),
                            op=mybir.AluOpType.mult,
                        )
                        nc.tensor.matmul(
                            out=ps3[:],
                            lhsT=mask,
                            rhs=ph2[:],
                            start=(g == 0),
                            stop=(g == NG - 1),
                        )
                    dn2 = ssb.tile([P, 1], F32, tag="dn2")
                    nc.vector.tensor_scalar_add(
                        out=dn2[:], in0=ps3[:, OC : OC + 1], scalar1=1e-16
                    )
                    rc2 = ssb.tile([P, 1], F32, tag="rc2")
                    nc.vector.reciprocal(out=rc2[:], in_=dn2[:])
                    outsb = ssb.tile([P, OC], F32, tag="outsb")
                    nc.vector.tensor_tensor(
                        out=outsb[:],
                        in0=ps3[:, 0:OC],
                        in1=rc2[:].to_broadcast([P, OC]),
                        op=mybir.AluOpType.mult,
                    )
                    if meta["b2_nonzero"]:
                        nc.vector.tensor_tensor(
                            out=outsb[:],
                            in0=outsb[:],
                            in1=b2_sb[:],
                            op=mybir.AluOpType.add,
                        )
                    nc.sync.dma_start(
                        out=out_d[t * P : (t + 1) * P, :], in_=outsb[:]
                    )

    nc.compile()
    return nc


def _default_cfg(n=25000, in_ch=256, hid=128, heads=4, out_ch=64, ng=None,
                 edge_index=None):
    if ng is None:
        # compute required NG from the edge data
        N = n
        ei = np.asarray(edge_index, np.int64)
        dst = np.concatenate([ei[1], np.arange(N, dtype=np.int64)])
        counts = np.bincount(dst // P, minlength=-(-n // (P * NCORES)) * NCORES)
        ng = int(np.ceil(counts.max() / P))
    return GATConfig(n, in_ch, hid, heads, out_ch, 0.2, ng)


def run(cfg, inputs, trace=False, tmpdir=None):
    in_maps, meta = host_prep(
        cfg,
        inputs["x"], inputs["edge_index"],
        inputs["W1"], inputs["a_src1"], inputs["a_dst1"], inputs["b1"],
        inputs["W2"], inputs["a_src2"], inputs["a_dst2"], inputs["b2"],
    )
    if meta["b1_nonzero"]:
        for m in in_maps:
            m["b1"] = np.tile(meta["b1"].reshape(1, -1), (P, 1))
    if meta["b2_nonzero"]:
        for m in in_maps:
            m["b2"] = np.tile(meta["b2"].reshape(1, -1), (P, 1))
    nc = build_program(cfg, meta)
    res = run_bass_kernel_spmd(
        nc,
        in_maps,
        core_ids=list(range(NCORES)),
        trace=trace,
        tmpdir=tmpdir,
    )
    shards = [res.results[c]["out"] for c in range(NCORES)]
    full = np.concatenate(shards, axis=0)[: cfg.N]
    return full, res


def kernel(**inputs):
    cfg = _default_cfg(
        n=inputs["x"].shape[0],
        in_ch=inputs["x"].shape[1],
        hid=inputs["a_src1"].shape[1],
        heads=inputs["a_src1"].shape[0],
        out_ch=inputs["a_src2"].shape[1],
        edge_index=inputs["edge_index"],
    )
    out, _ = run(cfg, inputs)
    return out.astype(np.float32)



# revision 11
# speedup vs baseline: 1.6031x; 1.6031x over previous
"""Two-layer GAT (PyG GATConv semantics) on 8 Trainium2 NeuronCores.

Strategy (graph/data parallel, per sharding hint):
  - Destination nodes are range-sharded across the 8 cores (3200 dsts/core,
    nodes padded 25000 -> 25600).  Node ids are RENUMBERED on the host so
    that every 128-dst tile carries a near-equal number of in-edges (LPT
    bin packing on in-degree).  This pins the per-tile edge-group count to
    its minimum and balances the cores.
  - Every core redundantly computes the full layer-1 node features
    h = x @ W1ext (W1ext also folds the attention projections: es = x @ (W1 @
    a_src), ed = x @ (W1 @ a_dst)), writes them as a row table in its own HBM.
  - Per dst tile the core issues ONE dma_gather for all NG*128 edge rows.
    Group 0 of every tile is the tile's own 128 rows (the appended
    self-loops): its gather indices are the diagonal, its mask is the
    identity (kept in SBUF), and its gathered es/ed tail doubles as the
    per-dst ed vector - no separate diagonal gather is needed.
  - Edge logits e = leakyrelu(es[src] + ed[dst]) and p = exp(e) are
    computed once per tile with batched strided ops over all groups.
    Softmax max-subtraction is skipped (shift invariance; logits are O(1)).
  - Numerator and denominator are accumulated with one-hot matmuls on the
    PE: numer += mask_g^T @ (p*h), denom += mask_g^T @ p.
  - Layer 2 repeats the same scheme on h2 = ELU(out1/denom) @ W2ext; the
    layer-2 row table is exchanged between cores with an AllGather, and the
    SAME gather-index/mask tables are reused (identical edge structure).

Row table layouts (bf16 elements):
  L1 row (640): [h (512)] | es float32 x4 (elems 512:520) | ed float32 x4
                (520:528) | pad (528:640)
  L2 row (128): [h2 (64) | es2 f32 (64:66) | ed2 f32 (66:68) | pad]
"""

import sys

for _p in ("/opt/trn_rl_repo",):
    if _p not in sys.path:
        sys.path.insert(0, _p)

import heapq

import numpy as np
import ml_dtypes

import concourse.bacc as bacc
import concourse.bass as bass
import concourse.mybir as mybir
import concourse.tile as tile
from concourse import library_config
from concourse._compat import axon_active
from concourse.bass_utils import run_bass_kernel_spmd
from concourse.masks import make_identity

BF16 = ml_dtypes.bfloat16
F32 = mybir.dt.float32
BF = mybir.dt.bfloat16
P = 128
NCORES = 8


class GATConfig:
    def __init__(self, n, in_ch, hid, heads, out_ch, neg_slope, ng):
        self.N = n
        self.NPAD = -(-n // (P * NCORES)) * (P * NCORES)
        self.SHARD = self.NPAD // NCORES
        self.T = self.SHARD // P              # dst tiles per core
        self.NT = self.NPAD // P              # node tiles (phase A)
        self.IN_CH = in_ch                    # must be multiple of 128
        self.KIN = in_ch // P
        self.HID = hid                        # 128
        self.HEADS = heads                    # 4
        self.OUT_CH = out_ch                  # 64
        self.NEG = neg_slope
        self.NG = ng                          # groups per dst tile (incl diag)
        self.H1 = heads * hid                 # 512
        # L1 row layout (bf16 elems): h | es f32 | ed f32 | pad
        self.R1_ES = self.H1                  # es f32 at elems 512:520
        self.R1_ED = self.H1 + 8              # ed f32 at elems 520:528
        self.ROW1 = -(-(self.H1 + 16) * 2 // 256) * 128
        # L2 row layout: h2 | es2 f32 | ed2 f32 | pad
        self.R2_ES = out_ch                   # es2 f32 at elems 64:66
        self.R2_ED = out_ch + 2               # ed2 f32 at elems 66:68
        self.ROW2 = -(-(out_ch + 8) * 2 // 256) * 128
        self.KH1 = self.H1 // P               # 4


def _wrap_idx(flat):
    """int16 flat index list -> [128, len/16] wrapped layout for dma_gather."""
    n = len(flat)
    assert n % 16 == 0
    w = np.asarray(flat, np.int16).reshape(n // 16, 16).T  # [16, n/16]
    return np.tile(w, (8, 1))                              # [128, n/16]


def _balance_tiles(indeg, npad, gtiles):
    """LPT bin-packing: assign each node to one of `gtiles` bins of exactly
    128 nodes, minimizing the max bin in-degree.  Returns new2old[npad]."""
    order = np.argsort(-indeg, kind="stable")
    heap = [(0, b) for b in range(gtiles)]
    heapq.heapify(heap)
    counts = np.zeros(gtiles, np.int32)
    members = [[] for _ in range(gtiles)]
    for nid in order:
        while True:
            load, b = heapq.heappop(heap)
            if counts[b] < P:
                break
        members[b].append(nid)
        counts[b] += 1
        if counts[b] < P:
            heapq.heappush(heap, (load + int(indeg[nid]), b))
    new2old = np.concatenate([np.array(m, np.int64) for m in members])
    assert new2old.shape[0] == npad
    return new2old


def host_prep(cfg, x, edge_index, W1, a_src1, a_dst1, b1, W2, a_src2, a_dst2, b2):
    """Build all per-core input arrays. Returns (in_maps, meta)."""
    N, NPAD = cfg.N, cfg.NPAD
    H, C, OC = cfg.HEADS, cfg.HID, cfg.OUT_CH

    # --- weights ---------------------------------------------------------
    W1 = np.asarray(W1, np.float32)
    W2 = np.asarray(W2, np.float32)
    a_src1 = np.asarray(a_src1, np.float32)
    a_dst1 = np.asarray(a_dst1, np.float32)
    a_src2 = np.asarray(a_src2, np.float32)
    a_dst2 = np.asarray(a_dst2, np.float32)
    w1ext = np.zeros((cfg.IN_CH, cfg.H1 + 8), np.float32)
    w1ext[:, : cfg.H1] = W1
    for h in range(H):
        w1ext[:, cfg.H1 + h] = W1[:, h * C : (h + 1) * C] @ a_src1[h]
        w1ext[:, cfg.H1 + 4 + h] = W1[:, h * C : (h + 1) * C] @ a_dst1[h]
    # -> [128, KIN, 520]
    w1eh = np.ascontiguousarray(
        w1ext.reshape(cfg.KIN, P, cfg.H1 + 8).transpose(1, 0, 2)
    ).astype(BF16)

    w2ext = np.zeros((cfg.H1, OC + 2), np.float32)
    w2ext[:, :OC] = W2
    w2ext[:, OC] = W2 @ a_src2[0]
    w2ext[:, OC + 1] = W2 @ a_dst2[0]
    w2eh = np.ascontiguousarray(
        w2ext.reshape(cfg.KH1, P, OC + 2).transpose(1, 0, 2)
    ).astype(BF16)

    # --- renumber nodes to balance per-tile in-degree ---------------------
    ei = np.asarray(edge_index, np.int64)
    gtiles = cfg.NT
    indeg = np.bincount(ei[1], minlength=NPAD)  # non-self in-degree
    new2old = _balance_tiles(indeg, NPAD, gtiles)
    old2new = np.empty(NPAD, np.int64)
    old2new[new2old] = np.arange(NPAD, dtype=np.int64)
    src_s = old2new[ei[0]].astype(np.int32)
    dst_s = old2new[ei[1]].astype(np.int32)
    order = np.argsort(dst_s, kind="stable")
    src_s = src_s[order]
    dst_s = dst_s[order]

    # --- x, permuted + transposed/tiled for lhsT --------------------------
    xp = np.zeros((NPAD, cfg.IN_CH), np.float32)
    xp[old2new[:N]] = np.asarray(x, np.float32)
    xth = np.ascontiguousarray(
        xp.reshape(cfg.NT, P, cfg.KIN, P).transpose(3, 0, 2, 1)
    ).reshape(P, cfg.NT * cfg.KIN * P).astype(BF16)

    # --- per-tile edge buckets --------------------------------------------
    counts = np.bincount(dst_s // P, minlength=gtiles)
    nge = int(np.ceil(counts.max() / P)) if counts.max() else 1
    assert nge + 1 <= cfg.NG, f"data needs NG={nge + 1} > configured {cfg.NG}"
    NG = cfg.NG
    NGE = NG - 1                       # edge groups (group 0 = diagonal)
    ET = NGE * P
    starts = np.concatenate([[0], np.cumsum(counts)])

    gidx = np.zeros((NCORES, cfg.T, P, NG * 8), np.int16)
    # mask layout: [core][128 slot-part, T, NGE, 256] (mask | maskT)
    maskpair = np.zeros((NCORES, P, cfg.T, NGE, 2 * P), BF16)
    eye = np.arange(P, dtype=np.int32)
    for gt in range(gtiles):
        c, t = divmod(gt, cfg.T)
        lo, hi = starts[gt], starts[gt + 1]
        k = hi - lo
        idx = np.zeros(NG * P, np.int32)
        idx[:P] = gt * P + eye                 # diagonal group (self-loops)
        idx[P : P + k] = src_s[lo:hi]
        gidx[c, t] = _wrap_idx(idx.astype(np.int16))
        dl = np.full(ET, -1, np.int32)
        dl[:k] = dst_s[lo:hi] - gt * P
        m = (dl.reshape(NGE, P, 1) == eye.reshape(1, 1, P)).astype(BF16)
        # maskpair[c, p, t, g, 0:128] = mask[slot (g,p), :]
        maskpair[c, :, t, :, :P] = m.transpose(1, 0, 2)
        # maskpair[c, p, t, g, 128:256] = maskT row p = mask[:, p]
        maskpair[c, :, t, :, P:] = m.transpose(2, 0, 1)

    in_maps = []
    for c in range(NCORES):
        in_maps.append(
            {
                "xth": xth,
                "w1eh": w1eh,
                "w2eh": w2eh,
                "gidx": gidx[c].reshape(cfg.T * P, NG * 8),
                "maskpair": maskpair[c].reshape(P, cfg.T * NGE * 2 * P),
            }
        )
    meta = {
        "b1_nonzero": bool(np.any(np.asarray(b1))),
        "b2_nonzero": bool(np.any(np.asarray(b2))),
        "b1": np.asarray(b1, np.float32),
        "b2": np.asarray(b2, np.float32),
        "new2old": new2old,
    }
    return in_maps, meta


def build_program(cfg, meta, phases="ABCD"):
    under_axon = axon_active()
    nc = bacc.Bacc(
        "TRN2",
        target_bir_lowering=False,
        debug=not under_axon,
        num_devices=NCORES,
        # one gather per dst tile = NG*128 descriptors in flight (plus the
        # next tile's); the carveout is per-partition SBUF, so keep it at
        # two calls' worth (2 * 17 * 128 descs * 16 B = 70 KB)
        dynamic_dma_scratch_size=65536,
    )
    H, C, OC, NG, T = cfg.HEADS, cfg.HID, cfg.OUT_CH, cfg.NG, cfg.T
    H1, KIN, KH1 = cfg.H1, cfg.KIN, cfg.KH1
    NGE = NG - 1
    ROW1, ROW2 = cfg.ROW1, cfg.ROW2
    F1 = ROW1 // 2                      # L1 row in f32 elems
    F2 = ROW2 // 2

    # I/O ------------------------------------------------------------------
    xth_d = nc.dram_tensor("xth", [P, cfg.NT * KIN * P], BF, kind="ExternalInput")
    w1eh_d = nc.dram_tensor("w1eh", [P, KIN, H1 + 8], BF, kind="ExternalInput")
    w2eh_d = nc.dram_tensor("w2eh", [P, KH1, OC + 2], BF, kind="ExternalInput")
    gidx_d = nc.dram_tensor("gidx", [T * P, NG * 8], mybir.dt.int16, kind="ExternalInput")
    maskp_d = nc.dram_tensor(
        "maskpair", [P, T * NGE * 2 * P], BF, kind="ExternalInput"
    )
    out_d = nc.dram_tensor("out", [cfg.SHARD, OC], F32, kind="ExternalOutput")

    table1 = nc.dram_tensor("table1", [cfg.NPAD, ROW1], BF)
    t2shard = nc.dram_tensor("t2shard", [cfg.SHARD, ROW2], BF)
    t2full = nc.dram_tensor("t2full", [cfg.NPAD, ROW2], BF, addr_space="Shared")

    if meta["b1_nonzero"]:
        b1_d = nc.dram_tensor("b1", [P, H1], F32, kind="ExternalInput")
    if meta["b2_nonzero"]:
        b2_d = nc.dram_tensor("b2", [P, OC], F32, kind="ExternalInput")

    with tile.TileContext(nc) as tc:
        nc.gpsimd.load_library(library_config.mlp)

        # persistent SBUF
        with tc.tile_pool(name="persist", bufs=1) as pp:
            w1eh = pp.tile([P, KIN, H1 + 8], BF)
            nc.sync.dma_start(out=w1eh[:], in_=w1eh_d[:])
            w2eh = pp.tile([P, KH1, OC + 2], BF)
            nc.sync.dma_start(out=w2eh[:], in_=w2eh_d[:])
            gidx = pp.tile([P, T, NG * 8], mybir.dt.int16)
            nc.sync.dma_start(
                out=gidx[:],
                in_=gidx_d[:].rearrange("(t p) s -> p t s", p=P),
            )
            ident = pp.tile([P, P], BF)
            make_identity(nc, ident[:])
            if meta["b1_nonzero"]:
                b1_sb = pp.tile([P, H1], F32)
                nc.sync.dma_start(out=b1_sb[:], in_=b1_d[:])
            if meta["b2_nonzero"]:
                b2_sb = pp.tile([P, OC], F32)
                nc.sync.dma_start(out=b2_sb[:], in_=b2_d[:])

            # ---------------- Phase A: h table ---------------------------
            if "A" in phases:
                WB = 8  # node tiles per table write
                with (
                    tc.tile_pool(name="xth_pool", bufs=2) as xp_pool,
                    tc.tile_pool(name="pa_sb", bufs=2) as pa_sb,
                    tc.tile_pool(name="pa_ps", bufs=2, space="PSUM") as pa_ps,
                ):
                    for nt0 in range(0, cfg.NT, WB):
                        xth = xp_pool.tile([P, WB * KIN * P], BF, tag="xth")
                        nc.sync.dma_start(
                            out=xth[:],
                            in_=xth_d[:, nt0 * KIN * P : (nt0 + WB) * KIN * P],
                        )
                        stage = pa_sb.tile([P, WB, ROW1], BF, tag="stage")
                        nc.vector.memset(stage[:, :, cfg.R1_ED + 8 :], 0.0)
                        for k in range(WB):
                            nt = nt0 + k
                            hps = pa_ps.tile([P, H1], F32, tag="hps")
                            eps = pa_ps.tile([P, 8], F32, tag="eps")
                            for j in range(KIN):
                                lhs = xth[:, (k * KIN + j) * P : (k * KIN + j + 1) * P]
                                nc.tensor.matmul(
                                    out=hps[:],
                                    lhsT=lhs,
                                    rhs=w1eh[:, j, 0:H1],
                                    start=(j == 0),
                                    stop=(j == KIN - 1),
                                )
                                nc.tensor.matmul(
                                    out=eps[:],
                                    lhsT=lhs,
                                    rhs=w1eh[:, j, H1 : H1 + 8],
                                    start=(j == 0),
                                    stop=(j == KIN - 1),
                                )
                            nc.scalar.activation(
                                out=stage[:, k, 0:H1],
                                in_=hps[:],
                                func=mybir.ActivationFunctionType.Copy,
                            )
                            nc.vector.tensor_copy(
                                out=stage[:, k, cfg.R1_ES : cfg.R1_ES + 16].bitcast(F32),
                                in_=eps[:],
                            )
                        nc.sync.dma_start(
                            out=table1[nt0 * P : (nt0 + WB) * P, :].rearrange(
                                "(t p) r -> p t r", p=P
                            ),
                            in_=stage[:],
                        )

            # ---------------- Phase B: L1 per dst-tile -------------------
            with (
                tc.tile_pool(name="gat_sb", bufs=2) as gsb,
                tc.tile_pool(name="mask_sb", bufs=2) as msb,
                tc.tile_pool(name="small_sb", bufs=3) as ssb,
                tc.tile_pool(name="acc_ps", bufs=2, space="PSUM") as aps,
                tc.tile_pool(name="sm_ps", bufs=2, space="PSUM") as gps,
                tc.tile_pool(name="tph_ps", bufs=1, space="PSUM") as tps,
            ):
                for t in range(T if "B" in phases else 0):
                    hg = gsb.tile([P, NG, ROW1], BF, tag="hg")
                    for c0 in range(0, NG, 8):
                        gch = min(8, NG - c0)
                        nc.gpsimd.dma_gather(
                            out_ap=hg[:, c0 : c0 + gch, :],
                            in_ap=table1[:],
                            idxs_ap=gidx[:, t, c0 * 8 : (c0 + gch) * 8],
                            num_idxs=gch * P,
                            num_idxs_reg=gch * P,
                            elem_size=ROW1,
                        )
                    mp = msb.tile([P, NGE, 2 * P], BF, tag="mp")
                    nc.sync.dma_start(
                        out=mp[:],
                        in_=maskp_d[:, t * NGE * 2 * P : (t + 1) * NGE * 2 * P],
                    )
                    hgf = hg[:].bitcast(F32)  # [128, NG, F1]
                    ES, ED = cfg.R1_ES // 2, cfg.R1_ED // 2
                    # per-dst ed vector from the diagonal group's tail
                    edbf = ssb.tile([P, H], BF, tag="edbf")
                    nc.vector.tensor_copy(out=edbf[:], in_=hgf[:, 0, ED : ED + H])
                    # broadcast ed[dst] to all edge slots: one small matmul/group
                    edps = gps.tile([P, NG, H], F32, tag="edps")
                    for g in range(NG):
                        lhsT = ident[:] if g == 0 else mp[:, g - 1, P : 2 * P]
                        nc.tensor.matmul(
                            out=edps[:, g, :], lhsT=lhsT, rhs=edbf[:],
                            start=True, stop=True,
                        )
                    # logits -> p, batched over all groups
                    elog = ssb.tile([P, NG, H], F32, tag="elog")
                    nc.vector.tensor_tensor(
                        out=elog[:], in0=hgf[:, :, ES : ES + H], in1=edps[:],
                        op=mybir.AluOpType.add,
                    )
                    esc = ssb.tile([P, NG, H], F32, tag="esc")
                    nc.vector.tensor_scalar_mul(out=esc[:], in0=elog[:], scalar1=cfg.NEG)
                    nc.vector.tensor_tensor(
                        out=elog[:], in0=elog[:], in1=esc[:], op=mybir.AluOpType.max
                    )
                    pbf = ssb.tile([P, NG, H], BF, tag="pbf")
                    nc.scalar.activation(
                        out=pbf[:], in_=elog[:], func=mybir.ActivationFunctionType.Exp
                    )
                    # p * h, batched per head
                    ph = gsb.tile([P, NG, H1], BF, tag="ph")
                    for h in range(H):
                        nc.vector.tensor_tensor(
                            out=ph[:, :, h * C : (h + 1) * C],
                            in0=hg[:, :, h * C : (h + 1) * C],
                            in1=pbf[:, :, h : h + 1].to_broadcast([P, NG, C]),
                            op=mybir.AluOpType.mult,
                        )
                    # accumulate numerator + denominator
                    nm = aps.tile([P, H1], F32, tag="nm")
                    dn = aps.tile([P, H], F32, tag="dn")
                    for g in range(NG):
                        lhsT = ident[:] if g == 0 else mp[:, g - 1, 0:P]
                        nc.tensor.matmul(
                            out=dn[:], lhsT=lhsT, rhs=pbf[:, g, :],
                            start=(g == 0), stop=(g == NG - 1),
                        )
                        nc.tensor.matmul(
                            out=nm[:], lhsT=lhsT, rhs=ph[:, g, :],
                            start=(g == 0), stop=(g == NG - 1),
                        )
                    dns = ssb.tile([P, H], F32, tag="dns")
                    nc.vector.tensor_scalar_add(out=dns[:], in0=dn[:], scalar1=1e-16)
                    rc = ssb.tile([P, H], F32, tag="rc")
                    nc.vector.reciprocal(out=rc[:], in_=dns[:])
                    v = gsb.tile([P, H1], F32, tag="v")
                    nc.vector.tensor_tensor(
                        out=v[:].rearrange("p (h c) -> p h c", c=C),
                        in0=nm[:].rearrange("p (h c) -> p h c", c=C),
                        in1=rc[:].to_broadcast([P, H, C]),
                        op=mybir.AluOpType.mult,
                    )
                    if meta["b1_nonzero"]:
                        nc.vector.tensor_tensor(
                            out=v[:], in0=v[:], in1=b1_sb[:], op=mybir.AluOpType.add
                        )
                    # ELU -> bf16
                    rneg = gsb.tile([P, H1], F32, tag="rneg")
                    nc.scalar.activation(
                        out=rneg[:], in_=v[:],
                        func=mybir.ActivationFunctionType.Relu, scale=-1.0,
                    )
                    sexp = gsb.tile([P, H1], F32, tag="sexp")
                    nc.scalar.activation(
                        out=sexp[:], in_=rneg[:],
                        func=mybir.ActivationFunctionType.Exp, scale=-1.0,
                    )
                    rpos = gsb.tile([P, H1], F32, tag="rpos")
                    nc.scalar.activation(
                        out=rpos[:], in_=v[:], func=mybir.ActivationFunctionType.Relu
                    )
                    nc.vector.tensor_tensor(
                        out=sexp[:], in0=rpos[:], in1=sexp[:], op=mybir.AluOpType.add
                    )
                    h2bf = gsb.tile([P, H1], BF, tag="h2bf")
                    nc.vector.tensor_scalar_add(out=h2bf[:], in0=sexp[:], scalar1=-1.0)
                    # transpose h2 -> 4x [128,128], then W2ext matmul
                    h2p = tps.tile([P, OC + 2], F32, tag="h2p")
                    for j in range(KH1):
                        tp = tps.tile([P, P], BF, tag="tp")
                        nc.tensor.transpose(
                            out=tp[:], in_=h2bf[:, j * P : (j + 1) * P], identity=ident[:]
                        )
                        h2t = ssb.tile([P, P], BF, tag="h2t")
                        nc.vector.tensor_copy(out=h2t[:], in_=tp[:])
                        nc.tensor.matmul(
                            out=h2p[:], lhsT=h2t[:], rhs=w2eh[:, j, :],
                            start=(j == 0), stop=(j == KH1 - 1),
                        )
                    stage2 = ssb.tile([P, ROW2], BF, tag="stage2")
                    nc.scalar.activation(
                        out=stage2[:, 0:OC], in_=h2p[:, 0:OC],
                        func=mybir.ActivationFunctionType.Copy,
                    )
                    nc.vector.tensor_copy(
                        out=stage2[:, cfg.R2_ES : cfg.R2_ES + 4].bitcast(F32),
                        in_=h2p[:, OC : OC + 2],
                    )
                    nc.vector.memset(stage2[:, cfg.R2_ES + 4 :], 0.0)
                    nc.sync.dma_start(
                        out=t2shard[t * P : (t + 1) * P, :], in_=stage2[:]
                    )

                # ---- all-gather the layer-2 table ----
                if "C" in phases:
                    nc.gpsimd.collective_compute(
                        "AllGather",
                        mybir.AluOpType.bypass,
                        replica_groups=[list(range(NCORES))],
                        ins=[t2shard.ap().opt()],
                        outs=[t2full.ap().opt()],
                    )

                # ---- layer 2 aggregation -> output ----
                ES2, ED2 = cfg.R2_ES // 2, cfg.R2_ED // 2
                for t in range(T if "D" in phases else 0):
                    hg2 = gsb.tile([P, NG, ROW2], BF, tag="hg2")
                    for c0 in range(0, NG, 8):
                        gch = min(8, NG - c0)
                        nc.gpsimd.dma_gather(
                            out_ap=hg2[:, c0 : c0 + gch, :],
                            in_ap=t2full[:],
                            idxs_ap=gidx[:, t, c0 * 8 : (c0 + gch) * 8],
                            num_idxs=gch * P,
                            num_idxs_reg=gch * P,
                            elem_size=ROW2,
                        )
                    mp = msb.tile([P, NGE, 2 * P], BF, tag="mp")
                    nc.sync.dma_start(
                        out=mp[:],
                        in_=maskp_d[:, t * NGE * 2 * P : (t + 1) * NGE * 2 * P],
                    )
                    hg2f = hg2[:].bitcast(F32)  # [128, NG, F2]
                    ed2bf = ssb.tile([P, 1], BF, tag="ed2bf")
                    nc.vector.tensor_copy(out=ed2bf[:], in_=hg2f[:, 0, ED2 : ED2 + 1])
                    edps2_t = gps.tile([P, NG, H], F32, tag="edps")
                    edps2 = edps2_t[:, :, 0:1]
                    for g in range(NG):
                        lhsT = ident[:] if g == 0 else mp[:, g - 1, P : 2 * P]
                        nc.tensor.matmul(
                            out=edps2[:, g, :], lhsT=lhsT, rhs=ed2bf[:],
                            start=True, stop=True,
                        )
                    elog2 = ssb.tile([P, NG, 1], F32, tag="elog2")
                    nc.vector.tensor_tensor(
                        out=elog2[:], in0=hg2f[:, :, ES2 : ES2 + 1], in1=edps2[:],
                        op=mybir.AluOpType.add,
                    )
                    esc2 = ssb.tile([P, NG, 1], F32, tag="esc2")
                    nc.vector.tensor_scalar_mul(
                        out=esc2[:], in0=elog2[:], scalar1=cfg.NEG
                    )
                    nc.vector.tensor_tensor(
                        out=elog2[:], in0=elog2[:], in1=esc2[:], op=mybir.AluOpType.max
                    )
                    p2bf = ssb.tile([P, NG, 1], BF, tag="p2bf")
                    nc.scalar.activation(
                        out=p2bf[:], in_=elog2[:], func=mybir.ActivationFunctionType.Exp
                    )
                    ph2 = ssb.tile([P, NG, OC], BF, tag="ph2")
                    nc.vector.tensor_tensor(
                        out=ph2[:],
                        in0=hg2[:, :, 0:OC],
                        in1=p2bf[:, :, 0:1].to_broadcast([P, NG, OC]),
                        op=mybir.AluOpType.mult,
                    )
                    nm2_t = aps.tile([P, H1], F32, tag="nm")
                    nm2 = nm2_t[:, 0:OC]
                    dn2_t = aps.tile([P, H], F32, tag="dn")
                    dn2 = dn2_t[:, 0:1]
                    for g in range(NG):
                        lhsT = ident[:] if g == 0 else mp[:, g - 1, 0:P]
                        nc.tensor.matmul(
                            out=dn2[:], lhsT=lhsT, rhs=p2bf[:, g, :],
                            start=(g == 0), stop=(g == NG - 1),
                        )
                        nc.tensor.matmul(
                            out=nm2[:], lhsT=lhsT, rhs=ph2[:, g, :],
                            start=(g == 0), stop=(g == NG - 1),
                        )
                    dns2 = ssb.tile([P, 1], F32, tag="dns2")
                    nc.vector.tensor_scalar_add(out=dns2[:], in0=dn2[:], scalar1=1e-16)
                    rc2 = ssb.tile([P, 1], F32, tag="rc2")
                    nc.vector.reciprocal(out=rc2[:], in_=dns2[:])
                    outsb = ssb.tile([P, OC], F32, tag="outsb")
                    nc.vector.tensor_tensor(
                        out=outsb[:],
                        in0=nm2[:],
                        in1=rc2[:].to_broadcast([P, OC]),
                        op=mybir.AluOpType.mult,
                    )
                    if meta["b2_nonzero"]:
                        nc.vector.tensor_tensor(
                            out=outsb[:], in0=outsb[:], in1=b2_sb[:],
                            op=mybir.AluOpType.add,
                        )
                    nc.sync.dma_start(
                        out=out_d[t * P : (t + 1) * P, :], in_=outsb[:]
                    )

    nc.compile()
    return nc


def _default_cfg(n=25000, in_ch=256, hid=128, heads=4, out_ch=64, ng=None,
                 edge_index=None):
    if ng is None:
        # NG = balanced edge groups + 1 diagonal group; run the same LPT
        # packing host_prep uses and read off the worst bin
        npad = (-(-n // (P * NCORES))) * P * NCORES
        gtiles = npad // P
        ei = np.asarray(edge_index, np.int64)
        indeg = np.bincount(ei[1], minlength=npad)
        new2old = _balance_tiles(indeg, npad, gtiles)
        old2new = np.empty(npad, np.int64)
        old2new[new2old] = np.arange(npad, dtype=np.int64)
        counts = np.bincount(old2new[ei[1]] // P, minlength=gtiles)
        ng = int(np.ceil(counts.max() / P)) + 1 if counts.max() else 2
    return GATConfig(n, in_ch, hid, heads, out_ch, 0.2, ng)


def run(cfg, inputs, trace=False, tmpdir=None):
    in_maps, meta = host_prep(
        cfg,
        inputs["x"], inputs["edge_index"],
        inputs["W1"], inputs["a_src1"], inputs["a_dst1"], inputs["b1"],
        inputs["W2"], inputs["a_src2"], inputs["a_dst2"], inputs["b2"],
    )
    if meta["b1_nonzero"]:
        for m in in_maps:
            m["b1"] = np.tile(meta["b1"].reshape(1, -1), (P, 1))
    if meta["b2_nonzero"]:
        for m in in_maps:
            m["b2"] = np.tile(meta["b2"].reshape(1, -1), (P, 1))
    nc = build_program(cfg, meta)
    res = run_bass_kernel_spmd(
        nc,
        in_maps,
        core_ids=list(range(NCORES)),
        trace=trace,
        tmpdir=tmpdir,
    )
    shards = [res.results[c]["out"] for c in range(NCORES)]
    full = np.concatenate(shards, axis=0)
    # un-permute: row i of `full` is node new2old[i]
    new2old = meta["new2old"]
    keep = new2old < cfg.N
    out = np.empty((cfg.N, cfg.OUT_CH), np.float32)
    out[new2old[keep]] = full[keep]
    return out, res


def kernel(**inputs):
    cfg = _default_cfg(
        n=inputs["x"].shape[0],
        in_ch=inputs["x"].shape[1],
        hid=inputs["a_src1"].shape[1],
        heads=inputs["a_src1"].shape[0],
        out_ch=inputs["a_src2"].shape[1],
        edge_index=inputs["edge_index"],
    )
    out, _ = run(cfg, inputs)
    return out.astype(np.float32)


# revision 19
# speedup vs baseline: 1.7169x; 1.0710x over previous
"""Two-layer GAT (PyG GATConv semantics) on 8 Trainium2 NeuronCores.

Strategy (graph/data parallel, per sharding hint):
  - Destination nodes are range-sharded across the 8 cores (3200 dsts/core,
    nodes padded 25000 -> 25600).  Node ids are RENUMBERED on the host so
    that every 128-dst tile carries a near-equal number of in-edges (LPT
    bin packing on in-degree).  This pins the per-tile edge-group count to
    its minimum and balances the cores.
  - Every core redundantly computes the full layer-1 node features
    h = x @ W1ext (W1ext also folds the attention projections: es = x @ (W1 @
    a_src), ed = x @ (W1 @ a_dst)), writes them as a row table in its own HBM.
  - Per dst tile the core issues ONE dma_gather for all NG*128 edge rows.
    Group 0 of every tile is the tile's own 128 rows (the appended
    self-loops): its gather indices are the diagonal, its mask is the
    identity (kept in SBUF), and its gathered es/ed tail doubles as the
    per-dst ed vector - no separate diagonal gather is needed.
  - Edge logits e = leakyrelu(es[src] + ed[dst]) and p = exp(e) are
    computed once per tile with batched strided ops over all groups.
    Softmax max-subtraction is skipped (shift invariance; logits are O(1)).
  - Numerator and denominator are accumulated with one-hot matmuls on the
    PE: numer += mask_g^T @ (p*h), denom += mask_g^T @ p.
  - Layer 2 repeats the same scheme on h2 = ELU(out1/denom) @ W2ext; the
    layer-2 row table is exchanged between cores with an AllGather, and the
    SAME gather-index/mask tables are reused (identical edge structure).

Row table layouts (bf16 elements):
  L1 row (640): [h (512)] | es float32 x4 (elems 512:520) | ed float32 x4
                (520:528) | pad (528:640)
  L2 row (128): [h2 (64) | es2 f32 (64:66) | ed2 f32 (66:68) | pad]
"""

import sys

for _p in ("/opt/trn_rl_repo",):
    if _p not in sys.path:
        sys.path.insert(0, _p)

import heapq

import numpy as np
import ml_dtypes

import concourse.bacc as bacc
import concourse.bass as bass
import concourse.mybir as mybir
import concourse.tile as tile
from concourse import library_config
from concourse._compat import axon_active
from concourse.bass_utils import run_bass_kernel_spmd
from concourse.masks import make_identity

BF16 = ml_dtypes.bfloat16
F32 = mybir.dt.float32
BF = mybir.dt.bfloat16
P = 128
NCORES = 8


class GATConfig:
    def __init__(self, n, in_ch, hid, heads, out_ch, neg_slope, ng):
        self.N = n
        self.NPAD = -(-n // (P * NCORES)) * (P * NCORES)
        self.SHARD = self.NPAD // NCORES
        self.T = self.SHARD // P              # dst tiles per core
        self.NT = self.NPAD // P              # node tiles (phase A)
        self.IN_CH = in_ch                    # must be multiple of 128
        self.KIN = in_ch // P
        self.HID = hid                        # 128
        self.HEADS = heads                    # 4
        self.OUT_CH = out_ch                  # 64
        self.NEG = neg_slope
        self.NG = ng                          # groups per dst tile (incl diag)
        self.H1 = heads * hid                 # 512
        # L1 row layout (bf16 elems): h | es f32 | ed f32 | pad
        self.R1_ES = self.H1                  # es f32 at elems 512:520
        self.R1_ED = self.H1 + 8              # ed f32 at elems 520:528
        self.ROW1 = -(-(self.H1 + 16) * 2 // 256) * 128
        # L2 row layout: h2 | es2 f32 | ed2 f32 | pad
        self.R2_ES = out_ch                   # es2 f32 at elems 64:66
        self.R2_ED = out_ch + 2               # ed2 f32 at elems 66:68
        self.ROW2 = -(-(out_ch + 8) * 2 // 256) * 128
        self.KH1 = self.H1 // P               # 4


def _wrap_idx(flat):
    """int16 flat index list -> [128, len/16] wrapped layout for dma_gather."""
    n = len(flat)
    assert n % 16 == 0
    w = np.asarray(flat, np.int16).reshape(n // 16, 16).T  # [16, n/16]
    return np.tile(w, (8, 1))                              # [128, n/16]


def _balance_tiles(indeg, npad, gtiles):
    """LPT bin-packing: assign each node to one of `gtiles` bins of exactly
    128 nodes, minimizing the max bin in-degree.  Returns new2old[npad]."""
    order = np.argsort(-indeg, kind="stable")
    heap = [(0, b) for b in range(gtiles)]
    heapq.heapify(heap)
    counts = np.zeros(gtiles, np.int32)
    members = [[] for _ in range(gtiles)]
    for nid in order:
        while True:
            load, b = heapq.heappop(heap)
            if counts[b] < P:
                break
        members[b].append(nid)
        counts[b] += 1
        if counts[b] < P:
            heapq.heappush(heap, (load + int(indeg[nid]), b))
    new2old = np.concatenate([np.array(m, np.int64) for m in members])
    assert new2old.shape[0] == npad
    return new2old


def host_prep(cfg, x, edge_index, W1, a_src1, a_dst1, b1, W2, a_src2, a_dst2, b2):
    """Build all per-core input arrays. Returns (in_maps, meta)."""
    N, NPAD = cfg.N, cfg.NPAD
    H, C, OC = cfg.HEADS, cfg.HID, cfg.OUT_CH

    # --- weights ---------------------------------------------------------
    W1 = np.asarray(W1, np.float32)
    W2 = np.asarray(W2, np.float32)
    a_src1 = np.asarray(a_src1, np.float32)
    a_dst1 = np.asarray(a_dst1, np.float32)
    a_src2 = np.asarray(a_src2, np.float32)
    a_dst2 = np.asarray(a_dst2, np.float32)
    w1ext = np.zeros((cfg.IN_CH, cfg.H1 + 8), np.float32)
    w1ext[:, : cfg.H1] = W1
    for h in range(H):
        w1ext[:, cfg.H1 + h] = W1[:, h * C : (h + 1) * C] @ a_src1[h]
        w1ext[:, cfg.H1 + 4 + h] = W1[:, h * C : (h + 1) * C] @ a_dst1[h]
    # -> [128, KIN, 520]
    w1eh = np.ascontiguousarray(
        w1ext.reshape(cfg.KIN, P, cfg.H1 + 8).transpose(1, 0, 2)
    ).astype(BF16)

    w2ext = np.zeros((cfg.H1, OC + 2), np.float32)
    w2ext[:, :OC] = W2
    w2ext[:, OC] = W2 @ a_src2[0]
    w2ext[:, OC + 1] = W2 @ a_dst2[0]
    w2eh = np.ascontiguousarray(
        w2ext.reshape(cfg.KH1, P, OC + 2).transpose(1, 0, 2)
    ).astype(BF16)

    # --- renumber nodes to balance per-tile in-degree ---------------------
    ei = np.asarray(edge_index, np.int64)
    gtiles = cfg.NT
    indeg = np.bincount(ei[1], minlength=NPAD)  # non-self in-degree
    new2old = _balance_tiles(indeg, NPAD, gtiles)
    old2new = np.empty(NPAD, np.int64)
    old2new[new2old] = np.arange(NPAD, dtype=np.int64)
    src_s = old2new[ei[0]].astype(np.int32)
    dst_s = old2new[ei[1]].astype(np.int32)
    order = np.argsort(dst_s, kind="stable")
    src_s = src_s[order]
    dst_s = dst_s[order]

    # --- x, permuted + transposed/tiled for lhsT --------------------------
    xp = np.zeros((NPAD, cfg.IN_CH), np.float32)
    xp[old2new[:N]] = np.asarray(x, np.float32)
    xth = np.ascontiguousarray(
        xp.reshape(cfg.NT, P, cfg.KIN, P).transpose(3, 0, 2, 1)
    ).reshape(P, cfg.NT * cfg.KIN * P).astype(BF16)

    # --- per-tile edge buckets --------------------------------------------
    counts = np.bincount(dst_s // P, minlength=gtiles)
    nge = int(np.ceil(counts.max() / P)) if counts.max() else 1
    assert nge + 1 <= cfg.NG, f"data needs NG={nge + 1} > configured {cfg.NG}"
    NG = cfg.NG
    NGE = NG - 1                       # edge groups (group 0 = diagonal)
    ET = NGE * P
    starts = np.concatenate([[0], np.cumsum(counts)])

    gidx = np.zeros((NCORES, cfg.T, P, NG * 8), np.int16)
    # mask layout: [core][128 slot-part, T, NGE, 256] (mask | maskT)
    maskpair = np.zeros((NCORES, P, cfg.T, NGE, 2 * P), BF16)
    eye = np.arange(P, dtype=np.int32)
    for gt in range(gtiles):
        c, t = divmod(gt, cfg.T)
        lo, hi = starts[gt], starts[gt + 1]
        k = hi - lo
        idx = np.zeros(NG * P, np.int32)
        idx[:P] = gt * P + eye                 # diagonal group (self-loops)
        idx[P : P + k] = src_s[lo:hi]
        gidx[c, t] = _wrap_idx(idx.astype(np.int16))
        dl = np.full(ET, -1, np.int32)
        dl[:k] = dst_s[lo:hi] - gt * P
        m = (dl.reshape(NGE, P, 1) == eye.reshape(1, 1, P)).astype(BF16)
        # maskpair[c, p, t, g, 0:128] = mask[slot (g,p), :]
        maskpair[c, :, t, :, :P] = m.transpose(1, 0, 2)
        # maskpair[c, p, t, g, 128:256] = maskT row p = mask[:, p]
        maskpair[c, :, t, :, P:] = m.transpose(2, 0, 1)

    in_maps = []
    for c in range(NCORES):
        in_maps.append(
            {
                "xth": xth,
                "w1eh": w1eh,
                "w2eh": w2eh,
                "gidx": gidx[c].reshape(cfg.T * P, NG * 8),
                "maskpair": maskpair[c].reshape(P, cfg.T * NGE * 2 * P),
            }
        )
    meta = {
        "b1_nonzero": bool(np.any(np.asarray(b1))),
        "b2_nonzero": bool(np.any(np.asarray(b2))),
        "b1": np.asarray(b1, np.float32),
        "b2": np.asarray(b2, np.float32),
        "new2old": new2old,
    }
    return in_maps, meta


def build_program(cfg, meta, phases="ABCD"):
    under_axon = axon_active()
    nc = bacc.Bacc(
        "TRN2",
        target_bir_lowering=False,
        debug=not under_axon,
        num_devices=NCORES,
        # gathers are chunked to <=1024 descriptors per call; the carveout
        # is per-partition SBUF, so 48 KiB keeps ~3 calls in flight
        dynamic_dma_scratch_size=49152,
    )
    H, C, OC, NG, T = cfg.HEADS, cfg.HID, cfg.OUT_CH, cfg.NG, cfg.T
    H1, KIN, KH1 = cfg.H1, cfg.KIN, cfg.KH1
    NGE = NG - 1
    ROW1, ROW2 = cfg.ROW1, cfg.ROW2
    F1 = ROW1 // 2                      # L1 row in f32 elems
    F2 = ROW2 // 2

    # I/O ------------------------------------------------------------------
    xth_d = nc.dram_tensor("xth", [P, cfg.NT * KIN * P], BF, kind="ExternalInput")
    w1eh_d = nc.dram_tensor("w1eh", [P, KIN, H1 + 8], BF, kind="ExternalInput")
    w2eh_d = nc.dram_tensor("w2eh", [P, KH1, OC + 2], BF, kind="ExternalInput")
    gidx_d = nc.dram_tensor("gidx", [T * P, NG * 8], mybir.dt.int16, kind="ExternalInput")
    maskp_d = nc.dram_tensor(
        "maskpair", [P, T * NGE * 2 * P], BF, kind="ExternalInput"
    )
    out_d = nc.dram_tensor("out", [cfg.SHARD, OC], F32, kind="ExternalOutput")

    table1 = nc.dram_tensor("table1", [cfg.NPAD, ROW1], BF)
    t2shard = nc.dram_tensor("t2shard", [cfg.SHARD, ROW2], BF)
    t2full = nc.dram_tensor("t2full", [cfg.NPAD, ROW2], BF, addr_space="Shared")

    if meta["b1_nonzero"]:
        b1_d = nc.dram_tensor("b1", [P, H1], F32, kind="ExternalInput")
    if meta["b2_nonzero"]:
        b2_d = nc.dram_tensor("b2", [P, OC], F32, kind="ExternalInput")

    with tile.TileContext(nc) as tc:
        nc.gpsimd.load_library(library_config.mlp)

        # persistent SBUF
        with tc.tile_pool(name="persist", bufs=1) as pp:
            w1eh = pp.tile([P, KIN, H1 + 8], BF)
            nc.sync.dma_start(out=w1eh[:], in_=w1eh_d[:])
            w2eh = pp.tile([P, KH1, OC + 2], BF)
            nc.sync.dma_start(out=w2eh[:], in_=w2eh_d[:])
            gidx = pp.tile([P, T, NG * 8], mybir.dt.int16)
            nc.sync.dma_start(
                out=gidx[:],
                in_=gidx_d[:].rearrange("(t p) s -> p t s", p=P),
            )
            ident = pp.tile([P, P], BF)
            make_identity(nc, ident[:])
            if meta["b1_nonzero"]:
                b1_sb = pp.tile([P, H1], F32)
                nc.sync.dma_start(out=b1_sb[:], in_=b1_d[:])
            if meta["b2_nonzero"]:
                b2_sb = pp.tile([P, OC], F32)
                nc.sync.dma_start(out=b2_sb[:], in_=b2_d[:])

            # ---------------- Phase A: h table ---------------------------
            if "A" in phases:
                WB = 8  # node tiles per table write
                with (
                    tc.tile_pool(name="xth_pool", bufs=2) as xp_pool,
                    tc.tile_pool(name="pa_sb", bufs=2) as pa_sb,
                    tc.tile_pool(name="pa_ps", bufs=2, space="PSUM") as pa_ps,
                ):
                    for nt0 in range(0, cfg.NT, WB):
                        xth = xp_pool.tile([P, WB * KIN * P], BF, tag="xth")
                        nc.sync.dma_start(
                            out=xth[:],
                            in_=xth_d[:, nt0 * KIN * P : (nt0 + WB) * KIN * P],
                        )
                        stage = pa_sb.tile([P, WB, ROW1], BF, tag="stage")
                        nc.vector.memset(stage[:, :, cfg.R1_ED + 8 :], 0.0)
                        for k in range(WB):
                            nt = nt0 + k
                            hps = pa_ps.tile([P, H1], F32, tag="hps")
                            eps = pa_ps.tile([P, 8], F32, tag="eps")
                            for j in range(KIN):
                                lhs = xth[:, (k * KIN + j) * P : (k * KIN + j + 1) * P]
                                nc.tensor.matmul(
                                    out=hps[:],
                                    lhsT=lhs,
                                    rhs=w1eh[:, j, 0:H1],
                                    start=(j == 0),
                                    stop=(j == KIN - 1),
                                )
                                nc.tensor.matmul(
                                    out=eps[:],
                                    lhsT=lhs,
                                    rhs=w1eh[:, j, H1 : H1 + 8],
                                    start=(j == 0),
                                    stop=(j == KIN - 1),
                                )
                            nc.scalar.activation(
                                out=stage[:, k, 0:H1],
                                in_=hps[:],
                                func=mybir.ActivationFunctionType.Copy,
                            )
                            nc.vector.tensor_copy(
                                out=stage[:, k, cfg.R1_ES : cfg.R1_ES + 16].bitcast(F32),
                                in_=eps[:],
                            )
                        nc.sync.dma_start(
                            out=table1[nt0 * P : (nt0 + WB) * P, :].rearrange(
                                "(t p) r -> p t r", p=P
                            ),
                            in_=stage[:],
                        )

            # ---------------- Phase B: L1 per dst-tile -------------------
            with (
                tc.tile_pool(name="hg_sb", bufs=3) as hgsb,
                tc.tile_pool(name="gat_sb", bufs=2) as gsb,
                tc.tile_pool(name="mask_sb", bufs=2) as msb,
                tc.tile_pool(name="small_sb", bufs=3) as ssb,
                tc.tile_pool(name="acc_ps", bufs=2, space="PSUM") as aps,
                tc.tile_pool(name="sm_ps", bufs=2, space="PSUM") as gps,
                tc.tile_pool(name="tph_ps", bufs=1, space="PSUM") as tps,
            ):
                for t in range(T if "B" in phases else 0):
                    hg = hgsb.tile([P, NG, ROW1], BF, tag="hg")
                    for c0 in range(0, NG, 8):
                        gch = min(8, NG - c0)
                        nc.gpsimd.dma_gather(
                            out_ap=hg[:, c0 : c0 + gch, :],
                            in_ap=table1[:],
                            idxs_ap=gidx[:, t, c0 * 8 : (c0 + gch) * 8],
                            num_idxs=gch * P,
                            num_idxs_reg=gch * P,
                            elem_size=ROW1,
                        )
                    mp = msb.tile([P, NGE, 2 * P], BF, tag="mp")
                    nc.scalar.dma_start(
                        out=mp[:],
                        in_=maskp_d[:, t * NGE * 2 * P : (t + 1) * NGE * 2 * P],
                    )
                    hgf = hg[:].bitcast(F32)  # [128, NG, F1]
                    ES, ED = cfg.R1_ES // 2, cfg.R1_ED // 2
                    # per-dst ed vector from the diagonal group's tail
                    edbf = ssb.tile([P, H], BF, tag="edbf")
                    nc.vector.tensor_copy(out=edbf[:], in_=hgf[:, 0, ED : ED + H])
                    # broadcast ed[dst] to all edge slots: one small matmul/group
                    edps = gps.tile([P, NG, H], F32, tag="edps")
                    for g in range(NG):
                        lhsT = ident[:] if g == 0 else mp[:, g - 1, P : 2 * P]
                        nc.tensor.matmul(
                            out=edps[:, g, :], lhsT=lhsT, rhs=edbf[:],
                            start=True, stop=True,
                        )
                    # logits -> p, batched over all groups
                    elog = ssb.tile([P, NG, H], F32, tag="elog")
                    nc.vector.tensor_tensor(
                        out=elog[:], in0=hgf[:, :, ES : ES + H], in1=edps[:],
                        op=mybir.AluOpType.add,
                    )
                    esc = ssb.tile([P, NG, H], F32, tag="esc")
                    nc.vector.tensor_scalar_mul(out=esc[:], in0=elog[:], scalar1=cfg.NEG)
                    nc.vector.tensor_tensor(
                        out=elog[:], in0=elog[:], in1=esc[:], op=mybir.AluOpType.max
                    )
                    pbf = ssb.tile([P, NG, H], BF, tag="pbf")
                    nc.scalar.activation(
                        out=pbf[:], in_=elog[:], func=mybir.ActivationFunctionType.Exp
                    )
                    # p * h, batched per head
                    ph = gsb.tile([P, NG, H1], BF, tag="ph")
                    for h in range(H):
                        nc.vector.tensor_tensor(
                            out=ph[:, :, h * C : (h + 1) * C],
                            in0=hg[:, :, h * C : (h + 1) * C],
                            in1=pbf[:, :, h : h + 1].to_broadcast([P, NG, C]),
                            op=mybir.AluOpType.mult,
                        )
                    # accumulate numerator + denominator
                    nm = aps.tile([P, H1], F32, tag="nm")
                    dn = aps.tile([P, H], F32, tag="dn")
                    for g in range(NG):
                        lhsT = ident[:] if g == 0 else mp[:, g - 1, 0:P]
                        nc.tensor.matmul(
                            out=dn[:], lhsT=lhsT, rhs=pbf[:, g, :],
                            start=(g == 0), stop=(g == NG - 1),
                        )
                        nc.tensor.matmul(
                            out=nm[:], lhsT=lhsT, rhs=ph[:, g, :],
                            start=(g == 0), stop=(g == NG - 1),
                        )
                    dns = ssb.tile([P, H], F32, tag="dns")
                    nc.vector.tensor_scalar_add(out=dns[:], in0=dn[:], scalar1=1e-16)
                    rc = ssb.tile([P, H], F32, tag="rc")
                    nc.vector.reciprocal(out=rc[:], in_=dns[:])
                    v = gsb.tile([P, H1], F32, tag="v")
                    nc.vector.tensor_tensor(
                        out=v[:].rearrange("p (h c) -> p h c", c=C),
                        in0=nm[:].rearrange("p (h c) -> p h c", c=C),
                        in1=rc[:].to_broadcast([P, H, C]),
                        op=mybir.AluOpType.mult,
                    )
                    if meta["b1_nonzero"]:
                        nc.vector.tensor_tensor(
                            out=v[:], in0=v[:], in1=b1_sb[:], op=mybir.AluOpType.add
                        )
                    # ELU -> bf16
                    rneg = gsb.tile([P, H1], F32, tag="rneg")
                    nc.scalar.activation(
                        out=rneg[:], in_=v[:],
                        func=mybir.ActivationFunctionType.Relu, scale=-1.0,
                    )
                    sexp = gsb.tile([P, H1], F32, tag="sexp")
                    nc.scalar.activation(
                        out=sexp[:], in_=rneg[:],
                        func=mybir.ActivationFunctionType.Exp, scale=-1.0,
                    )
                    rpos = gsb.tile([P, H1], F32, tag="rpos")
                    nc.scalar.activation(
                        out=rpos[:], in_=v[:], func=mybir.ActivationFunctionType.Relu
                    )
                    nc.vector.tensor_tensor(
                        out=sexp[:], in0=rpos[:], in1=sexp[:], op=mybir.AluOpType.add
                    )
                    h2bf = gsb.tile([P, H1], BF, tag="h2bf")
                    nc.vector.tensor_scalar_add(out=h2bf[:], in0=sexp[:], scalar1=-1.0)
                    # transpose h2 -> 4x [128,128], then W2ext matmul
                    h2p = tps.tile([P, OC + 2], F32, tag="h2p")
                    for j in range(KH1):
                        tp = tps.tile([P, P], BF, tag="tp")
                        nc.tensor.transpose(
                            out=tp[:], in_=h2bf[:, j * P : (j + 1) * P], identity=ident[:]
                        )
                        h2t = ssb.tile([P, P], BF, tag="h2t")
                        nc.vector.tensor_copy(out=h2t[:], in_=tp[:])
                        nc.tensor.matmul(
                            out=h2p[:], lhsT=h2t[:], rhs=w2eh[:, j, :],
                            start=(j == 0), stop=(j == KH1 - 1),
                        )
                    stage2 = ssb.tile([P, ROW2], BF, tag="stage2")
                    nc.scalar.activation(
                        out=stage2[:, 0:OC], in_=h2p[:, 0:OC],
                        func=mybir.ActivationFunctionType.Copy,
                    )
                    nc.vector.tensor_copy(
                        out=stage2[:, cfg.R2_ES : cfg.R2_ES + 4].bitcast(F32),
                        in_=h2p[:, OC : OC + 2],
                    )
                    nc.vector.memset(stage2[:, cfg.R2_ES + 4 :], 0.0)
                    nc.sync.dma_start(
                        out=t2shard[t * P : (t + 1) * P, :], in_=stage2[:]
                    )

                # ---- all-gather the layer-2 table ----
                if "C" in phases:
                    nc.gpsimd.collective_compute(
                        "AllGather",
                        mybir.AluOpType.bypass,
                        replica_groups=[list(range(NCORES))],
                        ins=[t2shard.ap().opt()],
                        outs=[t2full.ap().opt()],
                    )

                # ---- layer 2 aggregation -> output ----
                ES2, ED2 = cfg.R2_ES // 2, cfg.R2_ED // 2
                for t in range(T if "D" in phases else 0):
                    hg2 = hgsb.tile([P, NG, ROW2], BF, tag="hg2")
                    for c0 in range(0, NG, 8):
                        gch = min(8, NG - c0)
                        nc.gpsimd.dma_gather(
                            out_ap=hg2[:, c0 : c0 + gch, :],
                            in_ap=t2full[:],
                            idxs_ap=gidx[:, t, c0 * 8 : (c0 + gch) * 8],
                            num_idxs=gch * P,
                            num_idxs_reg=gch * P,
                            elem_size=ROW2,
                        )
                    mp = msb.tile([P, NGE, 2 * P], BF, tag="mp")
                    nc.scalar.dma_start(
                        out=mp[:],
                        in_=maskp_d[:, t * NGE * 2 * P : (t + 1) * NGE * 2 * P],
                    )
                    hg2f = hg2[:].bitcast(F32)  # [128, NG, F2]
                    ed2bf = ssb.tile([P, 1], BF, tag="ed2bf")
                    nc.vector.tensor_copy(out=ed2bf[:], in_=hg2f[:, 0, ED2 : ED2 + 1])
                    edps2_t = gps.tile([P, NG, H], F32, tag="edps")
                    edps2 = edps2_t[:, :, 0:1]
                    for g in range(NG):
                        lhsT = ident[:] if g == 0 else mp[:, g - 1, P : 2 * P]
                        nc.tensor.matmul(
                            out=edps2[:, g, :], lhsT=lhsT, rhs=ed2bf[:],
                            start=True, stop=True,
                        )
                    elog2 = ssb.tile([P, NG, 1], F32, tag="elog2")
                    nc.vector.tensor_tensor(
                        out=elog2[:], in0=hg2f[:, :, ES2 : ES2 + 1], in1=edps2[:],
                        op=mybir.AluOpType.add,
                    )
                    esc2 = ssb.tile([P, NG, 1], F32, tag="esc2")
                    nc.vector.tensor_scalar_mul(
                        out=esc2[:], in0=elog2[:], scalar1=cfg.NEG
                    )
                    nc.vector.tensor_tensor(
                        out=elog2[:], in0=elog2[:], in1=esc2[:], op=mybir.AluOpType.max
                    )
                    p2bf = ssb.tile([P, NG, 1], BF, tag="p2bf")
                    nc.scalar.activation(
                        out=p2bf[:], in_=elog2[:], func=mybir.ActivationFunctionType.Exp
                    )
                    # p2*h2 with p2 itself in the last column, so one matmul
                    # per group accumulates numerator AND denominator
                    ph2 = ssb.tile([P, NG, OC + 1], BF, tag="ph2")
                    nc.vector.tensor_tensor(
                        out=ph2[:, :, 0:OC],
                        in0=hg2[:, :, 0:OC],
                        in1=p2bf[:, :, 0:1].to_broadcast([P, NG, OC]),
                        op=mybir.AluOpType.mult,
                    )
                    nc.vector.tensor_copy(out=ph2[:, :, OC : OC + 1], in_=p2bf[:])
                    nm2_t = aps.tile([P, H1], F32, tag="nm")
                    nm2 = nm2_t[:, 0 : OC + 1]
                    for g in range(NG):
                        lhsT = ident[:] if g == 0 else mp[:, g - 1, 0:P]
                        nc.tensor.matmul(
                            out=nm2[:], lhsT=lhsT, rhs=ph2[:, g, :],
                            start=(g == 0), stop=(g == NG - 1),
                        )
                    dns2 = ssb.tile([P, 1], F32, tag="dns2")
                    nc.vector.tensor_scalar_add(
                        out=dns2[:], in0=nm2[:, OC : OC + 1], scalar1=1e-16
                    )
                    rc2 = ssb.tile([P, 1], F32, tag="rc2")
                    nc.vector.reciprocal(out=rc2[:], in_=dns2[:])
                    outsb = ssb.tile([P, OC], F32, tag="outsb")
                    nc.vector.tensor_tensor(
                        out=outsb[:],
                        in0=nm2[:, 0:OC],
                        in1=rc2[:].to_broadcast([P, OC]),
                        op=mybir.AluOpType.mult,
                    )
                    if meta["b2_nonzero"]:
                        nc.vector.tensor_tensor(
                            out=outsb[:], in0=outsb[:], in1=b2_sb[:],
                            op=mybir.AluOpType.add,
                        )
                    nc.sync.dma_start(
                        out=out_d[t * P : (t + 1) * P, :], in_=outsb[:]
                    )

    nc.compile()
    return nc


def _default_cfg(n=25000, in_ch=256, hid=128, heads=4, out_ch=64, ng=None,
                 edge_index=None):
    if ng is None:
        # NG = balanced edge groups + 1 diagonal group; run the same LPT
        # packing host_prep uses and read off the worst bin
        npad = (-(-n // (P * NCORES))) * P * NCORES
        gtiles = npad // P
        ei = np.asarray(edge_index, np.int64)
        indeg = np.bincount(ei[1], minlength=npad)
        new2old = _balance_tiles(indeg, npad, gtiles)
        old2new = np.empty(npad, np.int64)
        old2new[new2old] = np.arange(npad, dtype=np.int64)
        counts = np.bincount(old2new[ei[1]] // P, minlength=gtiles)
        ng = int(np.ceil(counts.max() / P)) + 1 if counts.max() else 2
    return GATConfig(n, in_ch, hid, heads, out_ch, 0.2, ng)


def run(cfg, inputs, trace=False, tmpdir=None):
    in_maps, meta = host_prep(
        cfg,
        inputs["x"], inputs["edge_index"],
        inputs["W1"], inputs["a_src1"], inputs["a_dst1"], inputs["b1"],
        inputs["W2"], inputs["a_src2"], inputs["a_dst2"], inputs["b2"],
    )
    if meta["b1_nonzero"]:
        for m in in_maps:
            m["b1"] = np.tile(meta["b1"].reshape(1, -1), (P, 1))
    if meta["b2_nonzero"]:
        for m in in_maps:
            m["b2"] = np.tile(meta["b2"].reshape(1, -1), (P, 1))
    nc = build_program(cfg, meta)
    res = run_bass_kernel_spmd(
        nc,
        in_maps,
        core_ids=list(range(NCORES)),
        trace=trace,
        tmpdir=tmpdir,
    )
    shards = [res.results[c]["out"] for c in range(NCORES)]
    full = np.concatenate(shards, axis=0)
    # un-permute: row i of `full` is node new2old[i]
    new2old = meta["new2old"]
    keep = new2old < cfg.N
    out = np.empty((cfg.N, cfg.OUT_CH), np.float32)
    out[new2old[keep]] = full[keep]
    return out, res


def kernel(**inputs):
    cfg = _default_cfg(
        n=inputs["x"].shape[0],
        in_ch=inputs["x"].shape[1],
        hid=inputs["a_src1"].shape[1],
        heads=inputs["a_src1"].shape[0],
        out_ch=inputs["a_src2"].shape[1],
        edge_index=inputs["edge_index"],
    )
    out, _ = run(cfg, inputs)
    return out.astype(np.float32)
